# revision 1
# baseline (speedup 1.0000x reference)
"""MixedQLinear (QUIK-style int4+fp16 outlier linear) on 8 TRN2 NeuronCores.

Sharding: token-parallel. x [4,2048,4096] -> 8192 tokens, 1024 per core;
weights replicated. Each core quantizes its tokens, runs the int4 GEMM
(exact in fp16 PE arithmetic: products of small ints accumulate exactly in
fp32 PSUM) plus the fp-outlier GEMM, dequantizes, and writes its [1024,4096]
slice of the output. Host concatenates.

Key algebra: with r = clip(round((x-mn)/scale),0,15) = q+8,
  out = (sum_k r*Wint) * scale * ws  +  mn*reduced_w  +  fp_x@Wfp^T + bias
(the -8 shift folds exactly against zero*reduced_w). mn*reduced_w and bias
ride as two extra contraction rows of the fp-outlier matmul.
"""

import numpy as np
import concourse.bass as bass
import concourse.tile as tile
import concourse.mybir as mybir
from concourse.bass_utils import run_bass_kernel_spmd
from bass_rust import ScopedClock, SyncInfo
from concourse.alu_op_type import AluOpType

# ---------------------------------------------------------------------------
# Workaround: this toolchain's walrus accepts at most one sync-wait on a
# TPB_CTRL (Drain) instruction; Tile's tail drain attaches one wait per
# active DMA queue. Split it into a chain of single-wait drains.
def _drain_and_barrier(self, tick_clock, wait_clock):
    drain_inst = self.nc.sync.drain()
    wait_clock.add_sem_waits(
        drain_inst.ins, ScopedClock({None: tick_clock.global_clock})
    )
    si = drain_inst.ins.sync_info
    ow = list(si.on_wait) if si is not None else []
    if len(ow) > 1:
        si.on_wait = [ow[0]]
        for w in ow[1:]:
            d2 = self.nc.sync.drain()
            d2.ins.sync_info = SyncInfo(on_wait=[w], on_update=[])
    self.nc.all_engine_barrier()
    assert self.sems is not None
    popped = self.nc._tile_sem_poison_stack.pop()
    assert popped is self._sem_poison
    self.nc.clear_and_free_semaphores(list(self.sems.allocated().values()))
    self.nc.all_engine_barrier()


tile.TileContext._drain_and_barrier = _drain_and_barrier


def _split_multiwait_instructions(nc):
    """Walrus here allows only one sync-wait per instruction: hoist extra
    waits onto same-engine NOPs inserted immediately before."""
    ctr = 0
    for fn in nc.m.functions:
        for bb in fn.blocks:
            insts = bb.instructions
            out = []
            changed = False
            for ins in insts:
                si = getattr(ins, "sync_info", None)
                ow = list(si.on_wait) if si is not None else []
                if len(ow) > 1:
                    changed = True
                    for w in ow[:-1]:
                        ctr += 1
                        out.append(
                            mybir.InstNoOp(
                                name=f"mwsplit-{ctr}",
                                sync_info=SyncInfo(on_wait=[w], on_update=[]),
                                engine=ins.engine,
                                bass_nofuse=True,
                            )
                        )
                    si.on_wait = [ow[-1]]
                out.append(ins)
            if changed:
                bb.instructions = out
# ---------------------------------------------------------------------------

N_CORES = 8
B, S, IN, OUT, FP = 4, 2048, 4096, 4096, 256
NT = (B * S) // N_CORES          # 1024 tokens per core
P = 128
KC = IN // P                     # 32 feature chunks
NOUT = 8                         # out-feature chunks
NSZ = OUT // NOUT                # 512
HT = 2                           # token halves (512) for quantize layout
HSZ = NT // HT                   # 512
TOKT = NT // P                   # 8 token tiles of 128
BIG = 30000.0

f16 = mybir.dt.float16
f32 = mybir.dt.float32
i8 = mybir.dt.int8

_prog_cache = {}


def _build_program():
    nc = bass.Bass()
    xs = nc.declare_dram_parameter("xs", [NT, IN], f16, isOutput=False)
    fpx = nc.declare_dram_parameter("fpx", [NT, FP], f16, isOutput=False)
    wint = nc.declare_dram_parameter("wint", [IN, OUT], f16, isOutput=False)
    wfp0 = nc.declare_dram_parameter("wfp0", [P, OUT], f16, isOutput=False)
    wfp1 = nc.declare_dram_parameter("wfp1", [P, OUT], f16, isOutput=False)
    wfp2 = nc.declare_dram_parameter("wfp2", [2, OUT], f16, isOutput=False)
    wsrow = nc.declare_dram_parameter("wsrow", [OUT], f16, isOutput=False)
    bmrow = nc.declare_dram_parameter("bmrow", [IN], f16, isOutput=False)
    rwrow = nc.declare_dram_parameter("rwrow", [OUT], f16, isOutput=False)
    out_d = nc.declare_dram_parameter("out", [NT, OUT], f16, isOutput=True)

    def bcast(ap, parts=P):
        # DRAM row -> all partitions: stride-0 partition dim, SWDGE DMA
        return bass.AP(
            tensor=ap.tensor, offset=ap.offset, ap=[[0, parts]] + list(ap.ap)
        )

    import contextlib
    with tile.TileContext(nc) as tc:
        with (
            tc.tile_pool(name="const", bufs=1) as cpool,
            tc.tile_pool(name="rt", bufs=1) as rtpool,
            tc.tile_pool(name="stat1", bufs=8) as s1pool,
            tc.tile_pool(name="psum", bufs=3, space="PSUM") as ppool,
            tc.tile_pool(name="dram", bufs=1, space="DRAM") as dpool,
        ):
            # ---- resident constants -------------------------------------
            bmB = cpool.tile([P, IN], f16, tag="bmB")
            nc.gpsimd.dma_start(bmB[:], bcast(bmrow[:]))
            wsB = cpool.tile([P, OUT], f16, tag="wsB")
            nc.gpsimd.dma_start(wsB[:], bcast(wsrow[:]))
            rwB = cpool.tile([P, OUT], f16, tag="rwB")
            nc.gpsimd.dma_start(rwB[:], bcast(rwrow[:]))
            wfp0_s = cpool.tile([P, OUT], f16, tag="wfp0")
            nc.sync.dma_start(wfp0_s[:], wfp0[:])
            wfp1_s = cpool.tile([P, OUT], f16, tag="wfp1")
            nc.sync.dma_start(wfp1_s[:], wfp1[:])
            wfp2_s = cpool.tile([2, OUT], f16, tag="wfp2")
            nc.sync.dma_start(wfp2_s[:], wfp2[:])

            # fp-outlier activations, transposed [feat, tok]
            fpt0 = cpool.tile([P, NT], f16, tag="fpt0")
            fpt1 = cpool.tile([P, NT], f16, tag="fpt1")
            fpt2 = cpool.tile([2, NT], f16, tag="fpt2")
            for h in range(HT):
                hs = slice(h * HSZ, (h + 1) * HSZ)
                nc.sync.dma_start_transpose(fpt0[:, hs], fpx[hs, 0:P])
                nc.sync.dma_start_transpose(fpt1[:, hs], fpx[hs, P:FP])
            nc.vector.memset(fpt2[:], 1.0)  # row1: ones (bias row)
            nc.vector.memset(fpt2[0:1, :], 0.0)  # row0 unused (zero*rw done on DVE)

            # DRAM staging for per-token stats rows
            mn32_d = dpool.tile([NT], f32, tag="mn32")
            inv32_d = dpool.tile([NT], f32, tag="inv32")

            # ---- phase S: per-token masked min/max -> scale/inv ---------
            scl = []  # per-tok-tile [128,1] f32 scale, kept for dequant
            zro = []  # per-tok-tile [128,1] f32 zero-point
            sctx = contextlib.ExitStack()
            spool = sctx.enter_context(tc.tile_pool(name="stat", bufs=2))
            for t in range(TOKT):
                ts_ = slice(t * P, (t + 1) * P)
                xtile = spool.tile([P, IN], f16, tag="xtile")
                nc.sync.dma_start(xtile[:], xs[ts_, :])
                scratch = spool.tile([P, IN], f16, tag="scratch")
                mn_t = s1pool.tile([P, 1], f32, tag=f"mn{t}")
                mx_t = s1pool.tile([P, 1], f32, tag=f"mx{t}")
                nc.vector.tensor_tensor(scratch[:], xtile[:], bmB[:], AluOpType.add)
                nc.vector.tensor_reduce(
                    mn_t[:], scratch[:], mybir.AxisListType.X, AluOpType.min
                )
                nc.vector.tensor_tensor(
                    scratch[:], xtile[:], bmB[:], AluOpType.subtract
                )
                nc.vector.tensor_reduce(
                    mx_t[:], scratch[:], mybir.AxisListType.X, AluOpType.max
                )
                sc_t = s1pool.tile([P, 1], f32, tag=f"sc{t}")
                nc.vector.tensor_tensor(sc_t[:], mx_t[:], mn_t[:], AluOpType.subtract)
                nc.vector.tensor_scalar(
                    sc_t[:], sc_t[:], 1.0 / 15.0, 1e-8,
                    AluOpType.mult, AluOpType.max,
                )
                inv_t = s1pool.tile([P, 1], f32, tag=f"inv{t}")
                nc.vector.reciprocal(inv_t[:], sc_t[:])
                # Newton step: inv *= (2 - scale*inv)  -> f32-accurate 1/scale
                nwt = s1pool.tile([P, 1], f32, tag=f"nw{t}")
                nc.vector.tensor_tensor(nwt[:], sc_t[:], inv_t[:], AluOpType.mult)
                nc.vector.tensor_scalar(
                    nwt[:], nwt[:], -1.0, 2.0, AluOpType.mult, AluOpType.add
                )
                nc.vector.tensor_tensor(inv_t[:], inv_t[:], nwt[:], AluOpType.mult)
                zr_t = s1pool.tile([P, 1], f32, name=f"zr{t}", tag=f"zr{t}")
                nc.vector.tensor_scalar(
                    zr_t[:], sc_t[:], 8.0, mn_t[:, 0:1],
                    AluOpType.mult, AluOpType.add,
                )
                nc.sync.dma_start(mn32_d[ts_], mn_t[:])
                nc.sync.dma_start(inv32_d[ts_], inv_t[:])
                scl.append(sc_t)
                zro.append(zr_t)
            sctx.close()

            # ---- phase Q: quantize into rt[k][feat, tok] ----------------
            qctx = contextlib.ExitStack()
            qpool = qctx.enter_context(tc.tile_pool(name="qw", bufs=3))
            bcpool = qctx.enter_context(tc.tile_pool(name="bc", bufs=2))
            rt = [
                [rtpool.tile([P, HSZ], f16, name=f"rt{k}_{h}", tag=f"rt{k}_{h}") for h in range(HT)]
                for k in range(KC)
            ]
            for h in range(HT):
                hs = slice(h * HSZ, (h + 1) * HSZ)
                mnB = bcpool.tile([P, HSZ], f32, tag="mnB")
                nc.gpsimd.dma_start(mnB[:], bcast(mn32_d[hs]))
                invB = bcpool.tile([P, HSZ], f32, tag="invB")
                nc.gpsimd.dma_start(invB[:], bcast(inv32_d[hs]))
                for k in range(KC):
                    xt = qpool.tile([P, HSZ], f16, tag="xt")
                    nc.sync.dma_start_transpose(
                        xt[:], xs[hs, k * P : (k + 1) * P]
                    )
                    q = qpool.tile([P, HSZ], f32, tag="q")
                    nc.vector.tensor_tensor(q[:], xt[:], mnB[:], AluOpType.subtract)
                    nc.vector.tensor_tensor(q[:], q[:], invB[:], AluOpType.mult)
                    r8 = qpool.tile([P, HSZ], i8, tag="r8")
                    nc.scalar.copy(r8[:], q[:])       # f32->i8 cast: round-half-even
                    nc.scalar.activation(
                        rt[k][h][:], r8[:],
                        mybir.ActivationFunctionType.Copy, bias=-8.0,
                    )  # i8->f16 with the -8 zero shift, exact

            # ---- phase M: matmuls + dequant -----------------------------
            qctx.close()
            mctx = contextlib.ExitStack()
            wpool = mctx.enter_context(tc.tile_pool(name="wp", bufs=2))
            dqpool = mctx.enter_context(tc.tile_pool(name="dq", bufs=2))
            KHALF = KC // 2
            for n in range(NOUT):
                ns = slice(n * NSZ, (n + 1) * NSZ)
                wn0 = wpool.tile([P, KHALF, NSZ], f16, name="wn0", tag="wn0")
                nc.sync.dma_start(
                    wn0[:],
                    wint[0 : KHALF * P, ns].rearrange("(k p) j -> p k j", p=P),
                )
                wn1 = wpool.tile([P, KHALF, NSZ], f16, name="wn1", tag="wn1")
                nc.sync.dma_start(
                    wn1[:],
                    wint[KHALF * P : IN, ns].rearrange("(k p) j -> p k j", p=P),
                )
                for t in range(TOKT):
                    h = t // (TOKT // HT)
                    tsl = slice((t % (TOKT // HT)) * P, (t % (TOKT // HT)) * P + P)
                    ts_ = slice(t * P, (t + 1) * P)
                    psum_i = ppool.tile([P, NSZ], f32, tag="pi")
                    for k in range(KC):
                        wk = wn0[:, k, :] if k < KHALF else wn1[:, k - KHALF, :]
                        nc.tensor.matmul(
                            psum_i[:], rt[k][h][:, tsl], wk,
                            start=(k == 0), stop=(k == KC - 1),
                        )
                    psum_f = ppool.tile([P, NSZ], f32, tag="pf")
                    nc.tensor.matmul(
                        psum_f[:], fpt0[:, ts_], wfp0_s[:, ns], start=True, stop=False
                    )
                    nc.tensor.matmul(
                        psum_f[:], fpt1[:, ts_], wfp1_s[:, ns], start=False, stop=False
                    )
                    nc.tensor.matmul(
                        psum_f[:], fpt2[:, ts_], wfp2_s[:, ns], start=False, stop=True
                    )
                    td = dqpool.tile([P, NSZ], f32, tag="td")
                    nc.scalar.activation(
                        td[:], psum_i[:], mybir.ActivationFunctionType.Copy,
                        scale=scl[t][:, 0:1],
                    )
                    nc.vector.tensor_tensor(td[:], td[:], wsB[:, ns], AluOpType.mult)
                    zc = dqpool.tile([P, NSZ], f32, tag="zc")
                    nc.vector.tensor_scalar(
                        zc[:], rwB[:, ns], zro[t][:, 0:1], None, AluOpType.mult
                    )
                    nc.vector.tensor_tensor(td[:], td[:], zc[:], AluOpType.add)
                    outt = dqpool.tile([P, NSZ], f16, tag="outt")
                    nc.vector.tensor_tensor(outt[:], td[:], psum_f[:], AluOpType.add)
                    nc.sync.dma_start(out_d[ts_, ns], outt[:])
            mctx.close()
    _split_multiwait_instructions(nc)
    return nc


def _get_program():
    if "nc" not in _prog_cache:
        _prog_cache["nc"] = _build_program()
    return _prog_cache["nc"]


def kernel(x, int_weight, fp_weight, bias, weights_scales, reduced_w,
           int_indices, fp_indices):
    x2 = np.asarray(x, dtype=np.float16).reshape(-1, IN)
    ii = np.asarray(int_indices).astype(np.int64)
    fi = np.asarray(fp_indices).astype(np.int64)

    wint_emb = np.zeros((IN, OUT), dtype=np.float16)
    wint_emb[ii, :] = np.asarray(int_weight).astype(np.float16).T
    wfp_all = np.ascontiguousarray(np.asarray(fp_weight, dtype=np.float16).T)
    wfp2 = np.stack([
        np.asarray(reduced_w, dtype=np.float16).reshape(-1),
        np.asarray(bias, dtype=np.float16).reshape(-1),
    ])
    wsrow = np.ascontiguousarray(
        np.asarray(weights_scales, dtype=np.float16).reshape(-1)
    )
    bmrow = np.zeros(IN, dtype=np.float16)
    bmrow[fi] = BIG

    nc = _get_program()
    in_maps = []
    for c in range(N_CORES):
        xsh = x2[c * NT : (c + 1) * NT]
        in_maps.append({
            "xs": np.ascontiguousarray(xsh),
            "fpx": np.ascontiguousarray(xsh[:, fi]),
            "wint": wint_emb,
            "wfp0": np.ascontiguousarray(wfp_all[0:P]),
            "wfp1": np.ascontiguousarray(wfp_all[P:FP]),
            "wfp2": wfp2,
            "wsrow": wsrow,
            "bmrow": bmrow,
            "rwrow": np.ascontiguousarray(
                np.asarray(reduced_w, dtype=np.float16).reshape(-1)
            ),
        })
    res = run_bass_kernel_spmd(nc, in_maps, list(range(N_CORES)))
    out = np.concatenate(
        [res.results[c]["out"] for c in range(N_CORES)], axis=0
    )
    return out.reshape(B, S, OUT).astype(np.float16)



# revision 3
# speedup vs baseline: 1.4068x; 1.4068x over previous
"""MixedQLinear (QUIK-style int4+fp16 outlier linear) on 8 TRN2 NeuronCores.

Sharding: token-parallel. x [4,2048,4096] -> 8192 tokens, 1024 per core;
weights replicated. Each core quantizes its tokens, runs the int4 GEMM in
fp8e4 DoubleRow mode (exact: int4 operands and their products are exactly
representable, fp32 PSUM accumulation of |sum|<2^24 is exact) plus the
fp16 outlier GEMM, dequantizes, and writes its [1024,4096] output slice.

Host-side prep is layout only: gather the 3840 int-feature columns,
pre-transpose activations, pre-swizzle weights, convert int4 weights to
fp8 bytes. All math (stats, quantize, GEMMs, dequant) runs on device.

Device schedule (per core), software-pipelined by token-half (512 tok):
  stats h0 -> quantize h0 -> [MM+dequant h0 || stats+quantize h1] -> MM h1
Engine split: PE int fp8-DoubleRow GEMM + fp16 outlier GEMM; Vector
min/max stats, quantize mult, dequant (fused (psum*scale)*ws then +fp);
Scalar round-to-i8 cast + i8->fp8 shift; GpSimd quantize subtract +
stat broadcasts. zero*reduced_w and bias ride the fp-outlier GEMM as two
extra contraction rows ([zero_t; 1] x [reduced_w_j; bias_j]).
"""

import numpy as np
import ml_dtypes
import concourse.bass as bass
import concourse.tile as tile
import concourse.mybir as mybir
from concourse.bass_utils import run_bass_kernel_spmd
from bass_rust import ScopedClock, SyncInfo
from concourse.alu_op_type import AluOpType

# ---------------------------------------------------------------------------
# Workaround: this toolchain's walrus accepts at most one sync-wait on a
# TPB_CTRL (Drain) instruction; Tile's tail drain attaches one wait per
# active DMA queue. Split it into a chain of single-wait drains.
def _drain_and_barrier(self, tick_clock, wait_clock):
    drain_inst = self.nc.sync.drain()
    wait_clock.add_sem_waits(
        drain_inst.ins, ScopedClock({None: tick_clock.global_clock})
    )
    si = drain_inst.ins.sync_info
    ow = list(si.on_wait) if si is not None else []
    if len(ow) > 1:
        si.on_wait = [ow[0]]
        for w in ow[1:]:
            d2 = self.nc.sync.drain()
            d2.ins.sync_info = SyncInfo(on_wait=[w], on_update=[])
    self.nc.all_engine_barrier()
    assert self.sems is not None
    popped = self.nc._tile_sem_poison_stack.pop()
    assert popped is self._sem_poison
    self.nc.clear_and_free_semaphores(list(self.sems.allocated().values()))
    self.nc.all_engine_barrier()


tile.TileContext._drain_and_barrier = _drain_and_barrier


def _split_multiwait_instructions(nc):
    """Walrus here allows only one sync-wait per instruction: hoist extra
    waits onto same-engine NOPs inserted immediately before."""
    ctr = 0
    for fn in nc.m.functions:
        for bb in fn.blocks:
            insts = bb.instructions
            out = []
            changed = False
            for ins in insts:
                si = getattr(ins, "sync_info", None)
                ow = list(si.on_wait) if si is not None else []
                if len(ow) > 1:
                    changed = True
                    for w in ow[:-1]:
                        ctr += 1
                        out.append(
                            mybir.InstNoOp(
                                name=f"mwsplit-{ctr}",
                                sync_info=SyncInfo(on_wait=[w], on_update=[]),
                                engine=ins.engine,
                                bass_nofuse=True,
                            )
                        )
                    si.on_wait = [ow[-1]]
                out.append(ins)
            if changed:
                bb.instructions = out
# ---------------------------------------------------------------------------

N_CORES = 8
B, S, IN, OUT, FP = 4, 2048, 4096, 4096, 256
INT = IN - FP                    # 3840 int-quantized features
NT = (B * S) // N_CORES          # 1024 tokens per core
P = 128
KI = INT // P                    # 30 int feature chunks
KP = KI // 2                     # 15 fp8 DoubleRow chunk pairs
NOUT = 8                         # out-feature slices
NSZ = OUT // NOUT                # 512
HT = 2                           # token halves for the pipeline
HSZ = NT // HT                   # 512
TPH = HSZ // P                   # 4 token tiles per half

f16 = mybir.dt.float16
f32 = mybir.dt.float32
f8 = mybir.dt.float8e4
i8 = mybir.dt.int8
DR = mybir.MatmulPerfMode.DoubleRow

_prog_cache = {}


def _build_program():
    nc = bass.Bass()
    xi = nc.declare_dram_parameter("xi", [NT, INT], f16, isOutput=False)
    xit = nc.declare_dram_parameter("xit", [INT, NT], f16, isOutput=False)
    fpxt = nc.declare_dram_parameter("fpxt", [FP, NT], f16, isOutput=False)
    w8 = nc.declare_dram_parameter("w8", [P, NOUT, KI, NSZ], f8, isOutput=False)
    wfp0 = nc.declare_dram_parameter("wfp0", [P, OUT], f16, isOutput=False)
    wfp1 = nc.declare_dram_parameter("wfp1", [P, OUT], f16, isOutput=False)
    wfp2 = nc.declare_dram_parameter("wfp2", [2, OUT], f16, isOutput=False)
    wsrow = nc.declare_dram_parameter("wsrow", [OUT], f16, isOutput=False)
    out_d = nc.declare_dram_parameter("out", [NT, OUT], f16, isOutput=True)

    def bcast(ap, parts=P):
        # DRAM row -> all partitions: stride-0 partition dim, SWDGE DMA
        return bass.AP(
            tensor=ap.tensor, offset=ap.offset, ap=[[0, parts]] + list(ap.ap)
        )

    with tile.TileContext(nc) as tc:
        with (
            tc.tile_pool(name="const", bufs=1) as cpool,
            tc.tile_pool(name="rt", bufs=1) as rtpool,
            tc.tile_pool(name="stat1", bufs=1) as s1pool,
            tc.tile_pool(name="sx", bufs=2) as spool,
            tc.tile_pool(name="bc", bufs=2) as bcpool,
            tc.tile_pool(name="qw", bufs=3) as qpool,
            tc.tile_pool(name="wp", bufs=2) as wpool,
            tc.tile_pool(name="dq", bufs=3) as dqpool,
            tc.tile_pool(name="psum", bufs=4, space="PSUM") as ppool,
            tc.tile_pool(name="dram", bufs=1, space="DRAM") as dpool,
        ):
            # ---- resident constants -------------------------------------
            wsB = cpool.tile([P, OUT], f16, tag="wsB")
            nc.gpsimd.dma_start(wsB[:], bcast(wsrow[:]))
            wfp0_s = cpool.tile([P, OUT], f16, tag="wfp0")
            nc.sync.dma_start(wfp0_s[:], wfp0[:])
            wfp1_s = cpool.tile([P, OUT], f16, tag="wfp1")
            nc.sync.dma_start(wfp1_s[:], wfp1[:])
            wfp2_s = cpool.tile([2, OUT], f16, tag="wfp2")
            nc.sync.dma_start(wfp2_s[:], wfp2[:])
            fpt0 = cpool.tile([P, NT], f16, tag="fpt0")
            nc.sync.dma_start(fpt0[:], fpxt[0:P, :])
            fpt1 = cpool.tile([P, NT], f16, tag="fpt1")
            nc.sync.dma_start(fpt1[:], fpxt[P:FP, :])
            # per-half [zero_t; ones] rows for the reduced_w/bias GEMM rows
            fpt2h = []
            for h in range(HT):
                t2 = cpool.tile([2, HSZ], f16, name=f"fpt2_{h}", tag=f"fpt2_{h}")
                nc.vector.memset(t2[:], 1.0)   # row 0 overwritten by zro DMA
                fpt2h.append(t2)

            # quantized activations: [feat128, pair2, tok] fp8 per (Kpair, half)
            rt = [
                [
                    rtpool.tile([P, 2, HSZ], f8, name=f"rt{k}_{h}", tag=f"rt{k}_{h}")
                    for h in range(HT)
                ]
                for k in range(KP)
            ]

            # DRAM staging rows for per-token stats
            mn32_d = dpool.tile([NT], f32, tag="mn32")
            inv32_d = dpool.tile([NT], f32, tag="inv32")
            zro16_d = dpool.tile([NT], f16, tag="zro16")

            scl = [None] * (HT * TPH)   # per-tile [128,1] f32 scale for dequant

            def emit_stats(t):
                ts_ = slice(t * P, (t + 1) * P)
                xst = spool.tile([P, INT], f16, tag="xst")
                nc.sync.dma_start(xst[:], xi[ts_, :])
                mn_t = s1pool.tile([P, 1], f32, name=f"mn{t}", tag=f"mn{t}")
                mx_t = s1pool.tile([P, 1], f32, name=f"mx{t}", tag=f"mx{t}")
                nc.vector.tensor_reduce(
                    mn_t[:], xst[:], mybir.AxisListType.X, AluOpType.min
                )
                nc.vector.tensor_reduce(
                    mx_t[:], xst[:], mybir.AxisListType.X, AluOpType.max
                )
                sc_t = s1pool.tile([P, 1], f32, name=f"sc{t}", tag=f"sc{t}")
                nc.vector.tensor_tensor(sc_t[:], mx_t[:], mn_t[:], AluOpType.subtract)
                nc.vector.tensor_scalar(
                    sc_t[:], sc_t[:], 1.0 / 15.0, 1e-8,
                    AluOpType.mult, AluOpType.max,
                )
                inv_t = s1pool.tile([P, 1], f32, name=f"inv{t}", tag=f"inv{t}")
                nc.vector.reciprocal(inv_t[:], sc_t[:])
                # Newton step: inv *= (2 - scale*inv) -> f32-accurate 1/scale
                nwt = s1pool.tile([P, 1], f32, name=f"nw{t}", tag=f"nw{t}")
                nc.vector.tensor_tensor(nwt[:], sc_t[:], inv_t[:], AluOpType.mult)
                nc.vector.tensor_scalar(
                    nwt[:], nwt[:], -1.0, 2.0, AluOpType.mult, AluOpType.add
                )
                nc.vector.tensor_tensor(inv_t[:], inv_t[:], nwt[:], AluOpType.mult)
                zr_t = s1pool.tile([P, 1], f32, name=f"zr{t}", tag=f"zr{t}")
                nc.vector.tensor_scalar(
                    zr_t[:], sc_t[:], 8.0, mn_t[:, 0:1],
                    AluOpType.mult, AluOpType.add,
                )
                zr16 = s1pool.tile([P, 1], f16, name=f"zr16_{t}", tag=f"zr16_{t}")
                nc.scalar.copy(zr16[:], zr_t[:])
                nc.sync.dma_start(mn32_d[ts_], mn_t[:])
                nc.sync.dma_start(inv32_d[ts_], inv_t[:])
                nc.sync.dma_start(zro16_d[ts_], zr16[:])
                scl[t] = sc_t

            qb = {}

            def emit_qsetup(h):
                hs = slice(h * HSZ, (h + 1) * HSZ)
                mnB = bcpool.tile([P, HSZ], f32, tag="mnB")
                nc.gpsimd.dma_start(mnB[:], bcast(mn32_d[hs]))
                invB = bcpool.tile([P, HSZ], f32, tag="invB")
                nc.gpsimd.dma_start(invB[:], bcast(inv32_d[hs]))
                nc.sync.dma_start(fpt2h[h][0:1, :], zro16_d[hs])
                qb[h] = (mnB, invB)

            def emit_quant(h, k):
                hs = slice(h * HSZ, (h + 1) * HSZ)
                mnB, invB = qb[h]
                xt = qpool.tile([P, HSZ], f16, tag="xt")
                nc.sync.dma_start(xt[:], xit[k * P : (k + 1) * P, hs])
                qf = qpool.tile([P, HSZ], f32, tag="qf")
                nc.gpsimd.tensor_tensor(qf[:], xt[:], mnB[:], AluOpType.subtract)
                nc.vector.tensor_tensor(qf[:], qf[:], invB[:], AluOpType.mult)
                r8 = qpool.tile([P, HSZ], i8, tag="r8")
                nc.scalar.copy(r8[:], qf[:])      # f32->i8: round-half-even
                nc.scalar.activation(
                    rt[k // 2][h][:, k % 2, :], r8[:],
                    mybir.ActivationFunctionType.Copy, bias=-8.0,
                )  # i8->fp8 with the -8 zero shift, exact

            def emit_mm_group(h, n, tl, wn):
                ns = slice(n * NSZ, (n + 1) * NSZ)
                t = h * TPH + tl
                ts_ = slice(t * P, (t + 1) * P)
                tsl = slice(tl * P, (tl + 1) * P)
                psum_i = ppool.tile([P, NSZ], f32, tag="pi")
                for k in range(KP):
                    nc.tensor.matmul(
                        psum_i[:], rt[k][h][:, :, tsl], wn[:, 2 * k : 2 * k + 2, :],
                        start=(k == 0), stop=(k == KP - 1), perf_mode=DR,
                    )
                psum_f = ppool.tile([P, NSZ], f32, tag="pf")
                nc.tensor.matmul(
                    psum_f[:], fpt0[:, ts_], wfp0_s[:, ns], start=True, stop=False
                )
                nc.tensor.matmul(
                    psum_f[:], fpt1[:, ts_], wfp1_s[:, ns], start=False, stop=False
                )
                nc.tensor.matmul(
                    psum_f[:], fpt2h[h][:, tsl], wfp2_s[:, ns], start=False, stop=True
                )
                return psum_i, psum_f

            def emit_dequant(h, n, tl, psum_i, psum_f):
                ns = slice(n * NSZ, (n + 1) * NSZ)
                t = h * TPH + tl
                ts_ = slice(t * P, (t + 1) * P)
                td = dqpool.tile([P, NSZ], f32, tag="td")
                nc.vector.scalar_tensor_tensor(
                    td[:], psum_i[:], scl[t][:, 0:1], wsB[:, ns],
                    AluOpType.mult, AluOpType.mult,
                )
                outt = dqpool.tile([P, NSZ], f16, tag="outt")
                nc.vector.tensor_tensor(outt[:], td[:], psum_f[:], AluOpType.add)
                nc.sync.dma_start(out_d[ts_, ns], outt[:])

            # ---- prologue: stats + quantize for half 0 ------------------
            for t in range(TPH):
                emit_stats(t)
            emit_qsetup(0)
            for k in range(KI):
                emit_quant(0, k)

            # ---- main: per half, MM+dequant; drip-feed half-1 prep ------
            for h in range(HT):
                for n in range(NOUT):
                    wn = wpool.tile([P, KI, NSZ], f8, tag="wn")
                    nc.sync.dma_start(wn[:], w8[:, n, :, :])
                    groups = [emit_mm_group(h, n, tl, wn) for tl in range(TPH)]
                    for tl, (pi, pf) in enumerate(groups):
                        emit_dequant(h, n, tl, pi, pf)
                    if h == 0:
                        if n < TPH:
                            emit_stats(TPH + n)
                        if n == 3:
                            emit_qsetup(1)
                        if n >= 3:
                            k0 = 6 * (n - 3)
                            for k in range(k0, min(k0 + 6, KI)):
                                emit_quant(1, k)
    _split_multiwait_instructions(nc)
    return nc


def _get_program():
    if "nc" not in _prog_cache:
        _prog_cache["nc"] = _build_program()
    return _prog_cache["nc"]


def prepare_in_maps(x, int_weight, fp_weight, bias, weights_scales, reduced_w,
                    int_indices, fp_indices):
    """Host-side layout prep shared by kernel() and the profiling harness."""
    x2 = np.asarray(x, dtype=np.float16).reshape(-1, IN)
    ii = np.asarray(int_indices).astype(np.int64)
    fi = np.asarray(fp_indices).astype(np.int64)

    xi_full = np.ascontiguousarray(x2[:, ii])          # [8192, 3840]
    fpx_full = np.ascontiguousarray(x2[:, fi])         # [8192, 256]

    # int4 weights -> fp8 bytes, pre-swizzled so each (partition, n-slice)
    # read is one contiguous 15KB line: w8[p, n, s, j] = Wt[s*128+p, n*512+j]
    wt = np.asarray(int_weight).astype(np.float32).T   # [3840, 4096]
    w8 = np.ascontiguousarray(
        wt.reshape(KI, P, NOUT, NSZ).transpose(1, 2, 0, 3)
    ).astype(ml_dtypes.float8_e4m3)                    # [128, 8, 30, 512]

    wfp_all = np.asarray(fp_weight, dtype=np.float16).T  # [256, 4096]
    wfp0 = np.ascontiguousarray(wfp_all[0:P])
    wfp1 = np.ascontiguousarray(wfp_all[P:FP])
    wfp2 = np.stack([
        np.asarray(reduced_w, dtype=np.float16).reshape(-1),
        np.asarray(bias, dtype=np.float16).reshape(-1),
    ])
    wsrow = np.ascontiguousarray(
        np.asarray(weights_scales, dtype=np.float16).reshape(-1)
    )

    in_maps = []
    for c in range(N_CORES):
        sl = slice(c * NT, (c + 1) * NT)
        xi = np.ascontiguousarray(xi_full[sl])
        in_maps.append({
            "xi": xi,
            "xit": np.ascontiguousarray(xi.T),
            "fpxt": np.ascontiguousarray(fpx_full[sl].T),
            "w8": w8,
            "wfp0": wfp0,
            "wfp1": wfp1,
            "wfp2": wfp2,
            "wsrow": wsrow,
        })
    return in_maps


def kernel(x, int_weight, fp_weight, bias, weights_scales, reduced_w,
           int_indices, fp_indices):
    in_maps = prepare_in_maps(
        x, int_weight, fp_weight, bias, weights_scales, reduced_w,
        int_indices, fp_indices,
    )
    nc = _get_program()
    res = run_bass_kernel_spmd(nc, in_maps, list(range(N_CORES)))
    out = np.concatenate(
        [res.results[c]["out"] for c in range(N_CORES)], axis=0
    )
    return out.reshape(B, S, OUT).astype(np.float16)


# revision 9
# speedup vs baseline: 1.5939x; 1.1330x over previous
"""MixedQLinear (QUIK-style int4+fp16 outlier linear) on 8 TRN2 NeuronCores.

Sharding: token-parallel. x [4,2048,4096] -> 8192 tokens, 1024 per core;
weights replicated. Each core quantizes its tokens, runs the int4 GEMM in
fp8e4 DoubleRow mode (exact: int4 operands and their products are exactly
representable, fp32 PSUM accumulation of |sum|<2^24 is exact) plus the
fp16 outlier GEMM, dequantizes, and writes its [1024,4096] output slice.

Host-side prep is layout only: gather the 3840 int-feature columns,
pre-transpose activations, pre-swizzle weights, convert int4 weights to
fp8 bytes. All math (stats, quantize, GEMMs, dequant) runs on device.

Device schedule (per core), software-pipelined by token-half (512 tok):
  stats h0 -> quantize h0 -> [MM+dequant h0 || stats+quantize h1] -> MM h1
Engine split: PE int fp8-DoubleRow GEMM + fp16 outlier GEMM; Vector
min/max stats, quantize mult, dequant (fused (psum*scale)*ws then +fp);
Scalar round-to-i8 cast + i8->fp8 shift; GpSimd quantize subtract +
stat broadcasts. zero*reduced_w and bias ride the fp-outlier GEMM as two
extra contraction rows ([zero_t; 1] x [reduced_w_j; bias_j]).
"""

import numpy as np
import ml_dtypes
import concourse.bass as bass
import concourse.tile as tile
import concourse.mybir as mybir
from concourse.bass_utils import run_bass_kernel_spmd
from bass_rust import ScopedClock, SyncInfo
from concourse.alu_op_type import AluOpType

# ---------------------------------------------------------------------------
# Workaround: this toolchain's walrus accepts at most one sync-wait on a
# TPB_CTRL (Drain) instruction; Tile's tail drain attaches one wait per
# active DMA queue. Split it into a chain of single-wait drains.
def _drain_and_barrier(self, tick_clock, wait_clock):
    drain_inst = self.nc.sync.drain()
    wait_clock.add_sem_waits(
        drain_inst.ins, ScopedClock({None: tick_clock.global_clock})
    )
    si = drain_inst.ins.sync_info
    ow = list(si.on_wait) if si is not None else []
    if len(ow) > 1:
        si.on_wait = [ow[0]]
        for w in ow[1:]:
            d2 = self.nc.sync.drain()
            d2.ins.sync_info = SyncInfo(on_wait=[w], on_update=[])
    self.nc.all_engine_barrier()
    assert self.sems is not None
    popped = self.nc._tile_sem_poison_stack.pop()
    assert popped is self._sem_poison
    self.nc.clear_and_free_semaphores(list(self.sems.allocated().values()))
    self.nc.all_engine_barrier()


tile.TileContext._drain_and_barrier = _drain_and_barrier


def _split_multiwait_instructions(nc):
    """Walrus here allows only one sync-wait per instruction: hoist extra
    waits onto same-engine NOPs inserted immediately before."""
    ctr = 0
    for fn in nc.m.functions:
        for bb in fn.blocks:
            insts = bb.instructions
            out = []
            changed = False
            for ins in insts:
                si = getattr(ins, "sync_info", None)
                ow = list(si.on_wait) if si is not None else []
                if len(ow) > 1:
                    changed = True
                    for w in ow[:-1]:
                        ctr += 1
                        out.append(
                            mybir.InstNoOp(
                                name=f"mwsplit-{ctr}",
                                sync_info=SyncInfo(on_wait=[w], on_update=[]),
                                engine=ins.engine,
                                bass_nofuse=True,
                            )
                        )
                    si.on_wait = [ow[-1]]
                out.append(ins)
            if changed:
                bb.instructions = out
# ---------------------------------------------------------------------------

N_CORES = 8
B, S, IN, OUT, FP = 4, 2048, 4096, 4096, 256
INT = IN - FP                    # 3840 int-quantized features
NT = (B * S) // N_CORES          # 1024 tokens per core
P = 128
KI = INT // P                    # 30 int feature chunks
KP = KI // 2                     # 15 fp8 DoubleRow chunk pairs
NOUT = 8                         # out-feature slices
NSZ = OUT // NOUT                # 512
HT = 2                           # token halves for the pipeline
HSZ = NT // HT                   # 512
TPH = HSZ // P                   # 4 token tiles per half

f16 = mybir.dt.float16
f32 = mybir.dt.float32
f8 = mybir.dt.float8e4
i8 = mybir.dt.int8
DR = mybir.MatmulPerfMode.DoubleRow

_prog_cache = {}


def _build_program():
    nc = bass.Bass()
    # stats input: int columns + duplicate of the first 256 int columns as
    # padding to an 8KB DMA line (min/max are unaffected by duplicates)
    xi = nc.declare_dram_parameter("xi", [NT, IN], f16, isOutput=False)
    # quantize input, pre-swizzled: xith[h, p, k, t] = x_int[h*512+t, k*128+p]
    xith = nc.declare_dram_parameter("xith", [HT, P, KI, HSZ], f16, isOutput=False)
    fpxt = nc.declare_dram_parameter("fpxt", [FP, NT], f16, isOutput=False)
    w8 = nc.declare_dram_parameter("w8", [P, NOUT, KI, NSZ], f8, isOutput=False)
    wfp0 = nc.declare_dram_parameter("wfp0", [P, OUT], f16, isOutput=False)
    wfp1 = nc.declare_dram_parameter("wfp1", [P, OUT], f16, isOutput=False)
    wfp2 = nc.declare_dram_parameter("wfp2", [2, OUT], f16, isOutput=False)
    wsrow = nc.declare_dram_parameter("wsrow", [OUT], f16, isOutput=False)
    out_d = nc.declare_dram_parameter("out", [NT, OUT], f16, isOutput=True)

    def bcast(ap, parts=P):
        # DRAM row -> all partitions: stride-0 partition dim, SWDGE DMA
        return bass.AP(
            tensor=ap.tensor, offset=ap.offset, ap=[[0, parts]] + list(ap.ap)
        )

    with tile.TileContext(nc) as tc:
        with (
            tc.tile_pool(name="const", bufs=1) as cpool,
            tc.tile_pool(name="rt", bufs=1) as rtpool,
            tc.tile_pool(name="stat1", bufs=1) as s1pool,
            tc.tile_pool(name="sx", bufs=2) as spool,
            tc.tile_pool(name="bc", bufs=2) as bcpool,
            tc.tile_pool(name="qw", bufs=3) as qpool,
            tc.tile_pool(name="wp", bufs=2) as wpool,
            tc.tile_pool(name="dq", bufs=3) as dqpool,
            tc.tile_pool(name="psum", bufs=4, space="PSUM") as ppool,
            tc.tile_pool(name="dram", bufs=1, space="DRAM") as dpool,
        ):
            # ---- resident constants -------------------------------------
            wsB = cpool.tile([P, OUT], f16, tag="wsB")
            nc.gpsimd.dma_start(wsB[:], bcast(wsrow[:]))
            wfp0_s = cpool.tile([P, OUT], f16, tag="wfp0")
            nc.sync.dma_start(wfp0_s[:], wfp0[:])
            wfp1_s = cpool.tile([P, OUT], f16, tag="wfp1")
            nc.sync.dma_start(wfp1_s[:], wfp1[:])
            wfp2_s = cpool.tile([2, OUT], f16, tag="wfp2")
            nc.sync.dma_start(wfp2_s[:], wfp2[:])
            fpt0 = cpool.tile([P, NT], f16, tag="fpt0")
            nc.sync.dma_start(fpt0[:], fpxt[0:P, :])
            fpt1 = cpool.tile([P, NT], f16, tag="fpt1")
            nc.sync.dma_start(fpt1[:], fpxt[P:FP, :])
            # per-half [zero_t; ones] rows for the reduced_w/bias GEMM rows
            fpt2h = []
            for h in range(HT):
                t2 = cpool.tile([2, HSZ], f16, name=f"fpt2_{h}", tag=f"fpt2_{h}")
                nc.vector.memset(t2[:], 1.0)   # row 0 overwritten by zro DMA
                fpt2h.append(t2)

            # quantized activations: [feat128, pair2, tok] fp8 per (Kpair, half)
            rt = [
                [
                    rtpool.tile([P, 2, HSZ], f8, name=f"rt{k}_{h}", tag=f"rt{k}_{h}")
                    for h in range(HT)
                ]
                for k in range(KP)
            ]

            # DRAM staging rows for per-token stats
            mn32_d = dpool.tile([NT], f32, tag="mn32")
            inv32_d = dpool.tile([NT], f32, tag="inv32")
            zro16_d = dpool.tile([NT], f16, tag="zro16")

            scl = [None] * (HT * TPH)   # per-tile [128,1] f32 scale for dequant

            # half-resident quantize activations, one big linear DMA each
            xith_s = []
            for h in range(HT):
                xs_h = rtpool.tile(
                    [P, KI, HSZ], f16, name=f"xith{h}", tag=f"xith{h}"
                )
                nc.sync.dma_start(xs_h[:], xith[h, :, :, :])
                xith_s.append(xs_h)

            def emit_stats(t):
                ts_ = slice(t * P, (t + 1) * P)
                xst = spool.tile([P, IN], f16, tag="xst")
                nc.sync.dma_start(xst[:], xi[ts_, :])
                mn_t = s1pool.tile([P, 1], f32, name=f"mn{t}", tag=f"mn{t}")
                mx_t = s1pool.tile([P, 1], f32, name=f"mx{t}", tag=f"mx{t}")
                nc.vector.tensor_reduce(
                    mn_t[:], xst[:], mybir.AxisListType.X, AluOpType.min
                )
                nc.vector.tensor_reduce(
                    mx_t[:], xst[:], mybir.AxisListType.X, AluOpType.max
                )
                sc_t = s1pool.tile([P, 1], f32, name=f"sc{t}", tag=f"sc{t}")
                nc.vector.tensor_tensor(sc_t[:], mx_t[:], mn_t[:], AluOpType.subtract)
                nc.vector.tensor_scalar(
                    sc_t[:], sc_t[:], 1.0 / 15.0, 1e-8,
                    AluOpType.mult, AluOpType.max,
                )
                inv_t = s1pool.tile([P, 1], f32, name=f"inv{t}", tag=f"inv{t}")
                nc.vector.reciprocal(inv_t[:], sc_t[:])
                # Newton step: inv *= (2 - scale*inv) -> f32-accurate 1/scale
                nwt = s1pool.tile([P, 1], f32, name=f"nw{t}", tag=f"nw{t}")
                nc.vector.tensor_tensor(nwt[:], sc_t[:], inv_t[:], AluOpType.mult)
                nc.vector.tensor_scalar(
                    nwt[:], nwt[:], -1.0, 2.0, AluOpType.mult, AluOpType.add
                )
                nc.vector.tensor_tensor(inv_t[:], inv_t[:], nwt[:], AluOpType.mult)
                zr_t = s1pool.tile([P, 1], f32, name=f"zr{t}", tag=f"zr{t}")
                nc.vector.tensor_scalar(
                    zr_t[:], sc_t[:], 8.0, mn_t[:, 0:1],
                    AluOpType.mult, AluOpType.add,
                )
                zr16 = s1pool.tile([P, 1], f16, name=f"zr16_{t}", tag=f"zr16_{t}")
                nc.scalar.copy(zr16[:], zr_t[:])
                nc.sync.dma_start(mn32_d[ts_], mn_t[:])
                nc.sync.dma_start(inv32_d[ts_], inv_t[:])
                nc.sync.dma_start(zro16_d[ts_], zr16[:])
                scl[t] = sc_t

            qb = {}

            def emit_qsetup(h):
                hs = slice(h * HSZ, (h + 1) * HSZ)
                mnB = bcpool.tile([P, HSZ], f32, tag="mnB")
                nc.gpsimd.dma_start(mnB[:], bcast(mn32_d[hs]))
                invB = bcpool.tile([P, HSZ], f32, tag="invB")
                nc.gpsimd.dma_start(invB[:], bcast(inv32_d[hs]))
                nc.sync.dma_start(fpt2h[h][0:1, :], zro16_d[hs])
                qb[h] = (mnB, invB)

            def emit_quant(h, k):
                mnB, invB = qb[h]
                xt = xith_s[h][:, k, :]
                qf = qpool.tile([P, HSZ], f32, tag="qf")
                nc.gpsimd.tensor_tensor(qf[:], xt, mnB[:], AluOpType.subtract)
                nc.vector.tensor_tensor(qf[:], qf[:], invB[:], AluOpType.mult)
                r8 = qpool.tile([P, HSZ], i8, tag="r8")
                nc.scalar.copy(r8[:], qf[:])      # f32->i8: round-half-even
                nc.scalar.activation(
                    rt[k // 2][h][:, k % 2, :], r8[:],
                    mybir.ActivationFunctionType.Copy, bias=-8.0,
                )  # i8->fp8 with the -8 zero shift, exact

            def emit_mm_group(h, n, tl, wn):
                ns = slice(n * NSZ, (n + 1) * NSZ)
                t = h * TPH + tl
                ts_ = slice(t * P, (t + 1) * P)
                tsl = slice(tl * P, (tl + 1) * P)
                psum_i = ppool.tile([P, NSZ], f32, tag="pi")
                for k in range(KP):
                    nc.tensor.matmul(
                        psum_i[:], rt[k][h][:, :, tsl], wn[:, 2 * k : 2 * k + 2, :],
                        start=(k == 0), stop=(k == KP - 1), perf_mode=DR,
                    )
                psum_f = ppool.tile([P, NSZ], f32, tag="pf")
                nc.tensor.matmul(
                    psum_f[:], fpt0[:, ts_], wfp0_s[:, ns], start=True, stop=False
                )
                nc.tensor.matmul(
                    psum_f[:], fpt1[:, ts_], wfp1_s[:, ns], start=False, stop=False
                )
                nc.tensor.matmul(
                    psum_f[:], fpt2h[h][:, tsl], wfp2_s[:, ns], start=False, stop=True
                )
                return psum_i, psum_f

            def emit_dequant(h, n, tl, psum_i, psum_f):
                ns = slice(n * NSZ, (n + 1) * NSZ)
                t = h * TPH + tl
                ts_ = slice(t * P, (t + 1) * P)
                td = dqpool.tile([P, NSZ], f32, tag="td")
                nc.vector.scalar_tensor_tensor(
                    td[:], psum_i[:], scl[t][:, 0:1], wsB[:, ns],
                    AluOpType.mult, AluOpType.mult,
                )
                outt = dqpool.tile([P, NSZ], f16, tag="outt")
                nc.vector.tensor_tensor(outt[:], td[:], psum_f[:], AluOpType.add)
                nc.sync.dma_start(out_d[ts_, ns], outt[:])

            # ---- prologue: stats + quantize for half 0 ------------------
            for t in range(TPH):
                emit_stats(t)
            emit_qsetup(0)
            for k in range(KI):
                emit_quant(0, k)

            # ---- main: per half, MM+dequant; drip-feed half-1 prep ------
            for h in range(HT):
                for n in range(NOUT):
                    wn = wpool.tile([P, KI, NSZ], f8, tag="wn")
                    nc.sync.dma_start(wn[:], w8[:, n, :, :])
                    groups = [emit_mm_group(h, n, tl, wn) for tl in range(TPH)]
                    for tl, (pi, pf) in enumerate(groups):
                        emit_dequant(h, n, tl, pi, pf)
                    if h == 0:
                        if n < TPH:
                            emit_stats(TPH + n)
                        if n == 3:
                            emit_qsetup(1)
                        if n >= 3:
                            k0 = 6 * (n - 3)
                            for k in range(k0, min(k0 + 6, KI)):
                                emit_quant(1, k)
    _split_multiwait_instructions(nc)
    return nc


def _get_program():
    if "nc" not in _prog_cache:
        _prog_cache["nc"] = _build_program()
    return _prog_cache["nc"]


def prepare_in_maps(x, int_weight, fp_weight, bias, weights_scales, reduced_w,
                    int_indices, fp_indices):
    """Host-side layout prep shared by kernel() and the profiling harness."""
    x2 = np.asarray(x, dtype=np.float16).reshape(-1, IN)
    ii = np.asarray(int_indices).astype(np.int64)
    fi = np.asarray(fp_indices).astype(np.int64)

    xi_full = np.ascontiguousarray(x2[:, ii])          # [8192, 3840]
    fpx_full = np.ascontiguousarray(x2[:, fi])         # [8192, 256]
    # stats input padded to 8KB rows with duplicated columns (DMA-friendly)
    xip_full = np.concatenate([xi_full, xi_full[:, : IN - INT]], axis=1)

    # int4 weights -> fp8 bytes, pre-swizzled so each (partition, n-slice)
    # read is one contiguous 15KB line: w8[p, n, s, j] = Wt[s*128+p, n*512+j]
    wt = np.asarray(int_weight).astype(np.float32).T   # [3840, 4096]
    w8 = np.ascontiguousarray(
        wt.reshape(KI, P, NOUT, NSZ).transpose(1, 2, 0, 3)
    ).astype(ml_dtypes.float8_e4m3)                    # [128, 8, 30, 512]

    wfp_all = np.asarray(fp_weight, dtype=np.float16).T  # [256, 4096]
    wfp0 = np.ascontiguousarray(wfp_all[0:P])
    wfp1 = np.ascontiguousarray(wfp_all[P:FP])
    wfp2 = np.stack([
        np.asarray(reduced_w, dtype=np.float16).reshape(-1),
        np.asarray(bias, dtype=np.float16).reshape(-1),
    ])
    wsrow = np.ascontiguousarray(
        np.asarray(weights_scales, dtype=np.float16).reshape(-1)
    )

    in_maps = []
    for c in range(N_CORES):
        sl = slice(c * NT, (c + 1) * NT)
        xi = xi_full[sl]
        # xith[h, p, k, t] = xi[h*512 + t, k*128 + p]
        xith = np.ascontiguousarray(
            xi.reshape(HT, HSZ, KI, P).transpose(0, 3, 2, 1)
        )
        in_maps.append({
            "xi": np.ascontiguousarray(xip_full[sl]),
            "xith": xith,
            "fpxt": np.ascontiguousarray(fpx_full[sl].T),
            "w8": w8,
            "wfp0": wfp0,
            "wfp1": wfp1,
            "wfp2": wfp2,
            "wsrow": wsrow,
        })
    return in_maps


def kernel(x, int_weight, fp_weight, bias, weights_scales, reduced_w,
           int_indices, fp_indices):
    in_maps = prepare_in_maps(
        x, int_weight, fp_weight, bias, weights_scales, reduced_w,
        int_indices, fp_indices,
    )
    nc = _get_program()
    res = run_bass_kernel_spmd(nc, in_maps, list(range(N_CORES)))
    out = np.concatenate(
        [res.results[c]["out"] for c in range(N_CORES)], axis=0
    )
    return out.reshape(B, S, OUT).astype(np.float16)


# revision 11
# speedup vs baseline: 1.6159x; 1.0138x over previous
"""MixedQLinear (QUIK-style int4+fp16 outlier linear) on 8 TRN2 NeuronCores.

Sharding: token-parallel. x [4,2048,4096] -> 8192 tokens, 1024 per core;
weights replicated. Each core quantizes its tokens, runs the int4 GEMM in
fp8e4 DoubleRow mode (exact: int4 operands and their products are exactly
representable, fp32 PSUM accumulation of |sum|<2^24 is exact) plus the
fp16 outlier GEMM, dequantizes, and writes its [1024,0:4096] output slice.

Host-side prep is layout only: gather the 3840 int-feature columns,
pre-swizzle activations and weights, convert int4 weights to fp8 bytes.
All math (stats, quantize, GEMMs, dequant) runs on device.

Device schedule (per core): token-tile-major software pipeline over 8
tiles of 128 tokens. All 8 fp8 weight slices stay SBUF-resident, so per
tile t: [PE: 8 out-slices x (15 DoubleRow int MMs + 3 fp16 MMs)] overlaps
[stats + quantize of tile t+1] on Vector/GpSimd/Scalar. First matmul
issues ~35us in; PE stays dense (HAM-warm) to the end.
Engine split: Vector min/max stats + quantize mult (fused round-to-i8
output) + dequant ((psum*scale)*ws, then +fp_psum); GpSimd quantize
subtract + stat broadcasts; Scalar i8->fp8 shift. zero*reduced_w and bias
ride the fp-outlier GEMM as two extra contraction rows.
"""

import numpy as np
import ml_dtypes
import concourse.bass as bass
import concourse.tile as tile
import concourse.mybir as mybir
from concourse.bass_utils import run_bass_kernel_spmd
from bass_rust import ScopedClock, SyncInfo
from concourse.alu_op_type import AluOpType

# ---------------------------------------------------------------------------
# Workaround: this toolchain's walrus accepts at most one sync-wait on a
# TPB_CTRL (Drain) instruction; Tile's tail drain attaches one wait per
# active DMA queue. Split it into a chain of single-wait drains.
def _drain_and_barrier(self, tick_clock, wait_clock):
    drain_inst = self.nc.sync.drain()
    wait_clock.add_sem_waits(
        drain_inst.ins, ScopedClock({None: tick_clock.global_clock})
    )
    si = drain_inst.ins.sync_info
    ow = list(si.on_wait) if si is not None else []
    if len(ow) > 1:
        si.on_wait = [ow[0]]
        for w in ow[1:]:
            d2 = self.nc.sync.drain()
            d2.ins.sync_info = SyncInfo(on_wait=[w], on_update=[])
    self.nc.all_engine_barrier()
    assert self.sems is not None
    popped = self.nc._tile_sem_poison_stack.pop()
    assert popped is self._sem_poison
    self.nc.clear_and_free_semaphores(list(self.sems.allocated().values()))
    self.nc.all_engine_barrier()


tile.TileContext._drain_and_barrier = _drain_and_barrier


def _split_multiwait_instructions(nc):
    """Walrus here allows only one sync-wait per instruction: hoist extra
    waits onto same-engine NOPs inserted immediately before."""
    ctr = 0
    for fn in nc.m.functions:
        for bb in fn.blocks:
            insts = bb.instructions
            out = []
            changed = False
            for ins in insts:
                si = getattr(ins, "sync_info", None)
                ow = list(si.on_wait) if si is not None else []
                if len(ow) > 1:
                    changed = True
                    for w in ow[:-1]:
                        ctr += 1
                        out.append(
                            mybir.InstNoOp(
                                name=f"mwsplit-{ctr}",
                                sync_info=SyncInfo(on_wait=[w], on_update=[]),
                                engine=ins.engine,
                                bass_nofuse=True,
                            )
                        )
                    si.on_wait = [ow[-1]]
                out.append(ins)
            if changed:
                bb.instructions = out
# ---------------------------------------------------------------------------

N_CORES = 8
B, S, IN, OUT, FP = 4, 2048, 4096, 4096, 256
INT = IN - FP                    # 3840 int-quantized features
NT = (B * S) // N_CORES          # 1024 tokens per core
P = 128
KI = INT // P                    # 30 int feature chunks
KIP = 32                         # chunks padded to an 8KB DMA line
KP = KI // 2                     # 15 fp8 DoubleRow chunk pairs
NOUT = 8                         # out-feature slices
NSZ = OUT // NOUT                # 512
TOKT = NT // P                   # 8 token tiles

f16 = mybir.dt.float16
f32 = mybir.dt.float32
f8 = mybir.dt.float8e4
i8 = mybir.dt.int8
DR = mybir.MatmulPerfMode.DoubleRow

_prog_cache = {}


def _build_program():
    nc = bass.Bass()
    # stats input: int columns + duplicate of the first 256 int columns as
    # padding to an 8KB DMA line (min/max are unaffected by duplicates)
    xi = nc.declare_dram_parameter("xi", [NT, IN], f16, isOutput=False)
    # quantize input, per-tile swizzle: xq[t, p, k, j] = x_int[t*128+j, k*128+p]
    xq_d = nc.declare_dram_parameter("xq", [TOKT, P, KIP, P], f16, isOutput=False)
    fpxt = nc.declare_dram_parameter("fpxt", [FP, NT], f16, isOutput=False)
    w8 = nc.declare_dram_parameter("w8", [P, NOUT, KI, NSZ], f8, isOutput=False)
    wfp0 = nc.declare_dram_parameter("wfp0", [P, OUT], f16, isOutput=False)
    wfp1 = nc.declare_dram_parameter("wfp1", [P, OUT], f16, isOutput=False)
    wfp2 = nc.declare_dram_parameter("wfp2", [2, OUT], f16, isOutput=False)
    wsrow = nc.declare_dram_parameter("wsrow", [OUT], f16, isOutput=False)
    out_d = nc.declare_dram_parameter("out", [NT, OUT], f16, isOutput=True)

    def bcast(ap, parts=P):
        # DRAM row -> all partitions: stride-0 partition dim, SWDGE DMA
        return bass.AP(
            tensor=ap.tensor, offset=ap.offset, ap=[[0, parts]] + list(ap.ap)
        )

    with tile.TileContext(nc) as tc:
        with (
            tc.tile_pool(name="const", bufs=1) as cpool,
            tc.tile_pool(name="rt", bufs=2) as rtpool,
            tc.tile_pool(name="stat1", bufs=1) as s1pool,
            tc.tile_pool(name="sx", bufs=2) as xpool,
            tc.tile_pool(name="bc", bufs=2) as bcpool,
            tc.tile_pool(name="qw", bufs=2) as qpool,
            tc.tile_pool(name="dq", bufs=2) as dqpool,
            tc.tile_pool(name="psum", bufs=4, space="PSUM") as ppool,
            tc.tile_pool(name="dram", bufs=1, space="DRAM") as dpool,
        ):
            # DRAM staging rows for per-token stats
            mn32_d = dpool.tile([NT], f32, tag="mn32")
            inv32_d = dpool.tile([NT], f32, tag="inv32")
            zro16_d = dpool.tile([NT], f16, tag="zro16")

            scl = [None] * TOKT     # per-tile [128,1] f32 scale for dequant
            xqs = [None] * TOKT     # per-tile quantize input tiles
            qb = [None] * TOKT      # per-tile (mnB, invB) broadcast tiles
            rts = [None] * TOKT     # per-tile list of 15 fp8 pair tiles

            def emit_xdma(t):
                ts_ = slice(t * P, (t + 1) * P)
                xst = xpool.tile([P, IN], f16, tag="xst")
                nc.sync.dma_start(xst[:], xi[ts_, :])
                xqt = xpool.tile([P, KIP, P], f16, tag="xq")
                nc.sync.dma_start(xqt[:], xq_d[t, :, :, :])
                xqs[t] = (xst, xqt)

            def emit_stats_min(t):
                xst, _ = xqs[t]
                mn_t = s1pool.tile([P, 1], f32, name=f"mn{t}", tag=f"mn{t}")
                nc.vector.tensor_reduce(
                    mn_t[:], xst[:], mybir.AxisListType.X, AluOpType.min
                )
                return mn_t

            def emit_stats_rest(t, mn_t):
                ts_ = slice(t * P, (t + 1) * P)
                xst, _ = xqs[t]
                mx_t = s1pool.tile([P, 1], f32, name=f"mx{t}", tag=f"mx{t}")
                nc.vector.tensor_reduce(
                    mx_t[:], xst[:], mybir.AxisListType.X, AluOpType.max
                )
                sc_t = s1pool.tile([P, 1], f32, name=f"sc{t}", tag=f"sc{t}")
                nc.vector.tensor_tensor(sc_t[:], mx_t[:], mn_t[:], AluOpType.subtract)
                nc.vector.tensor_scalar(
                    sc_t[:], sc_t[:], 1.0 / 15.0, 1e-8,
                    AluOpType.mult, AluOpType.max,
                )
                inv_t = s1pool.tile([P, 1], f32, name=f"inv{t}", tag=f"inv{t}")
                nc.vector.reciprocal(inv_t[:], sc_t[:])
                # Newton step: inv *= (2 - scale*inv) -> f32-accurate 1/scale
                nwt = s1pool.tile([P, 1], f32, name=f"nw{t}", tag=f"nw{t}")
                nc.vector.tensor_tensor(nwt[:], sc_t[:], inv_t[:], AluOpType.mult)
                nc.vector.tensor_scalar(
                    nwt[:], nwt[:], -1.0, 2.0, AluOpType.mult, AluOpType.add
                )
                nc.vector.tensor_tensor(inv_t[:], inv_t[:], nwt[:], AluOpType.mult)
                zr_t = s1pool.tile([P, 1], f32, name=f"zr{t}", tag=f"zr{t}")
                nc.vector.tensor_scalar(
                    zr_t[:], sc_t[:], 8.0, mn_t[:, 0:1],
                    AluOpType.mult, AluOpType.add,
                )
                zr16 = s1pool.tile([P, 1], f16, name=f"zr16_{t}", tag=f"zr16_{t}")
                nc.scalar.copy(zr16[:], zr_t[:])
                nc.sync.dma_start(mn32_d[ts_], mn_t[:])
                nc.sync.dma_start(inv32_d[ts_], inv_t[:])
                nc.sync.dma_start(zro16_d[ts_], zr16[:])
                scl[t] = sc_t
                # broadcast rows across partitions for the quantize phase
                mnB = bcpool.tile([P, P], f32, tag="mnB")
                nc.gpsimd.dma_start(mnB[:], bcast(mn32_d[ts_]))
                invB = bcpool.tile([P, P], f32, tag="invB")
                nc.gpsimd.dma_start(invB[:], bcast(inv32_d[ts_]))
                nc.sync.dma_start(fpt2t[t][0:1, :], zro16_d[ts_])
                qb[t] = (mnB, invB)
                rts[t] = [
                    rtpool.tile([P, 2, P], f8, name=f"rt{k}_{t}", tag=f"rt{k}")
                    for k in range(KP)
                ]

            def emit_quant(t, k):
                mnB, invB = qb[t]
                xt = xqs[t][1][:, k, :]
                qf = qpool.tile([P, P], f32, tag="qf")
                nc.gpsimd.tensor_tensor(qf[:], xt, mnB[:], AluOpType.subtract)
                r8 = qpool.tile([P, P], i8, tag="r8")
                # fused round: f32 mult with i8 output casts round-half-even
                nc.vector.tensor_tensor(r8[:], qf[:], invB[:], AluOpType.mult)
                nc.scalar.activation(
                    rts[t][k // 2][:, k % 2, :], r8[:],
                    mybir.ActivationFunctionType.Copy, bias=-8.0,
                )  # i8->fp8 with the -8 zero shift, exact

            def emit_mm_group(t, n):
                ns = slice(n * NSZ, (n + 1) * NSZ)
                ts_ = slice(t * P, (t + 1) * P)
                psum_i = ppool.tile([P, NSZ], f32, tag="pi")
                for k in range(KP):
                    nc.tensor.matmul(
                        psum_i[:], rts[t][k][:], wn[n][:, 2 * k : 2 * k + 2, :],
                        start=(k == 0), stop=(k == KP - 1), perf_mode=DR,
                    )
                psum_f = ppool.tile([P, NSZ], f32, tag="pf")
                nc.tensor.matmul(
                    psum_f[:], fpt0[:, ts_], wfp0_s[:, ns], start=True, stop=False
                )
                nc.tensor.matmul(
                    psum_f[:], fpt1[:, ts_], wfp1_s[:, ns], start=False, stop=False
                )
                nc.tensor.matmul(
                    psum_f[:], fpt2t[t][:], wfp2_s[:, ns], start=False, stop=True
                )
                return psum_i, psum_f

            def emit_dequant(t, n, psum_i, psum_f):
                ns = slice(n * NSZ, (n + 1) * NSZ)
                ts_ = slice(t * P, (t + 1) * P)
                td = dqpool.tile([P, NSZ], f16, tag="td")
                nc.vector.scalar_tensor_tensor(
                    td[:], psum_i[:], scl[t][:, 0:1], wsB[:, ns],
                    AluOpType.mult, AluOpType.mult,
                )
                outt = dqpool.tile([P, NSZ], f16, tag="outt")
                nc.vector.tensor_tensor(outt[:], td[:], psum_f[:], AluOpType.add)
                nc.sync.dma_start(out_d[ts_, ns], outt[:])

            # ---- prologue ----------------------------------------------
            # activation DMAs for tile 0 first so stats/quantize start ASAP
            fpt2t = [
                cpool.tile([2, P], f16, name=f"fpt2_{t}", tag=f"fpt2_{t}")
                for t in range(TOKT)
            ]
            emit_xdma(0)
            for t in range(TOKT):
                nc.vector.memset(fpt2t[t][:], 1.0)  # row 0 overwritten by zro

            wsB = cpool.tile([P, OUT], f16, tag="wsB")
            nc.gpsimd.dma_start(wsB[:], bcast(wsrow[:]))
            wfp0_s = cpool.tile([P, OUT], f16, tag="wfp0")
            nc.sync.dma_start(wfp0_s[:], wfp0[:])
            wfp1_s = cpool.tile([P, OUT], f16, tag="wfp1")
            nc.sync.dma_start(wfp1_s[:], wfp1[:])
            wfp2_s = cpool.tile([2, OUT], f16, tag="wfp2")
            nc.sync.dma_start(wfp2_s[:], wfp2[:])
            fpt0 = cpool.tile([P, NT], f16, tag="fpt0")
            nc.sync.dma_start(fpt0[:], fpxt[0:P, :])
            fpt1 = cpool.tile([P, NT], f16, tag="fpt1")
            nc.sync.dma_start(fpt1[:], fpxt[P:FP, :])

            # resident fp8 weight slices; n=0,1 queued ahead of the rest
            wn = [None] * NOUT
            for n in range(NOUT):
                wn[n] = cpool.tile([P, KI, NSZ], f8, name=f"wn{n}", tag=f"wn{n}")
            nc.sync.dma_start(wn[0][:], w8[:, 0, :, :])
            nc.sync.dma_start(wn[1][:], w8[:, 1, :, :])

            mn0 = emit_stats_min(0)
            emit_stats_rest(0, mn0)
            for k in range(KI):
                emit_quant(0, k)

            # ---- main: per token tile, MMs overlap next tile's prep ------
            mn_next = None
            for t in range(TOKT):
                tn = t + 1
                for n in range(NOUT):
                    if t == 0 and n < 6:
                        nc.sync.dma_start(wn[n + 2][:], w8[:, n + 2, :, :])
                    pi, pf = emit_mm_group(t, n)
                    emit_dequant(t, n, pi, pf)
                    if tn < TOKT:
                        if n == 0:
                            emit_xdma(tn)
                        elif n == 1:
                            mn_next = emit_stats_min(tn)
                        elif n == 2:
                            emit_stats_rest(tn, mn_next)
                        elif n >= 3:
                            k0 = 8 * (n - 3)
                            for k in range(k0, min(k0 + 8, KI)):
                                emit_quant(tn, k)
    _split_multiwait_instructions(nc)
    return nc


def _get_program():
    if "nc" not in _prog_cache:
        _prog_cache["nc"] = _build_program()
    return _prog_cache["nc"]


def prepare_in_maps(x, int_weight, fp_weight, bias, weights_scales, reduced_w,
                    int_indices, fp_indices):
    """Host-side layout prep shared by kernel() and the profiling harness."""
    x2 = np.asarray(x, dtype=np.float16).reshape(-1, IN)
    ii = np.asarray(int_indices).astype(np.int64)
    fi = np.asarray(fp_indices).astype(np.int64)

    xi_full = np.ascontiguousarray(x2[:, ii])          # [8192, 3840]
    fpx_full = np.ascontiguousarray(x2[:, fi])         # [8192, 256]
    # stats input padded to 8KB rows with duplicated columns (DMA-friendly)
    xip_full = np.concatenate([xi_full, xi_full[:, : IN - INT]], axis=1)

    # int4 weights -> fp8 bytes, pre-swizzled so each (partition, n-slice)
    # read is one contiguous 15KB line: w8[p, n, s, j] = Wt[s*128+p, n*512+j]
    wt = np.asarray(int_weight).astype(np.float32).T   # [3840, 4096]
    w8 = np.ascontiguousarray(
        wt.reshape(KI, P, NOUT, NSZ).transpose(1, 2, 0, 3)
    ).astype(ml_dtypes.float8_e4m3)                    # [128, 8, 30, 512]

    wfp_all = np.asarray(fp_weight, dtype=np.float16).T  # [256, 4096]
    wfp0 = np.ascontiguousarray(wfp_all[0:P])
    wfp1 = np.ascontiguousarray(wfp_all[P:FP])
    wfp2 = np.stack([
        np.asarray(reduced_w, dtype=np.float16).reshape(-1),
        np.asarray(bias, dtype=np.float16).reshape(-1),
    ])
    wsrow = np.ascontiguousarray(
        np.asarray(weights_scales, dtype=np.float16).reshape(-1)
    )

    in_maps = []
    for c in range(N_CORES):
        sl = slice(c * NT, (c + 1) * NT)
        xi = xi_full[sl]
        # xq[t, p, k, j] = xi[t*128 + j, k*128 + p], k padded 30 -> 32
        xq = np.zeros((TOKT, P, KIP, P), dtype=np.float16)
        xq[:, :, :KI, :] = xi.reshape(TOKT, P, KI, P).transpose(0, 3, 2, 1)
        in_maps.append({
            "xi": np.ascontiguousarray(xip_full[sl]),
            "xq": xq,
            "fpxt": np.ascontiguousarray(fpx_full[sl].T),
            "w8": w8,
            "wfp0": wfp0,
            "wfp1": wfp1,
            "wfp2": wfp2,
            "wsrow": wsrow,
        })
    return in_maps


def kernel(x, int_weight, fp_weight, bias, weights_scales, reduced_w,
           int_indices, fp_indices):
    in_maps = prepare_in_maps(
        x, int_weight, fp_weight, bias, weights_scales, reduced_w,
        int_indices, fp_indices,
    )
    nc = _get_program()
    res = run_bass_kernel_spmd(nc, in_maps, list(range(N_CORES)))
    out = np.concatenate(
        [res.results[c]["out"] for c in range(N_CORES)], axis=0
    )
    return out.reshape(B, S, OUT).astype(np.float16)
